# revision 15
# baseline (speedup 1.0000x reference)
"""Multi-relational GAT (2-layer encoder + 2-layer decoder) on 8 TRN2 NeuronCores.

Row-parallel over the N=2048 nodes (256 rows/core). Per GAT layer:
  1. each core computes Wh_local = x_local @ W[r] (bf16 matmuls, fp32 psum)
     and f12T_local = [W@a1 | W@a2]^T @ x_local (fp32r, attention logits),
  2. AllGather f12T (tiny, fp32) and Wh (bf16, one collective for all 4
     relations) across the 8 cores,
  3. each core builds its transposed attention tile PT[m, n] =
     exp(leakyrelu(f1[n] + f2[m])) * adjT[m, n] (ACT Prelu+Exp, GpSimd mask
     multiply) and accumulates out[n, d] = sum_m PT[m, n] * Wh[m, d] plus
     row sums s[n] on the PE,
  4. normalize by 1/s, ELU, (LayerNorm), maxpool over the 4 relations.
The question-gating front end and the tiny 5-choice head run on host numpy.
"""
import os
import numpy as np

N_CORES = 8
N, D, R, NLAYER = 2048, 512, 4, 4
NL = N // N_CORES          # 256 local rows per core
NT = N // 128              # 16 m-tiles of 128
KJ = D // 128              # 4 k-tiles over the feature dim
ALPHA = 0.2
EPS = 1e-5

_PROGRAMS = {}


def _build_program(ln_trivial):
    import concourse.bacc as bacc
    import concourse.mybir as mybir
    import concourse.tile as tile
    from concourse import masks
    from contextlib import ExitStack

    FP32 = mybir.dt.float32
    FP32R = mybir.dt.float32r
    BF16 = mybir.dt.bfloat16
    AF = mybir.ActivationFunctionType
    ALU = mybir.AluOpType
    RG = [list(range(N_CORES))]

    nc = bacc.Bacc("TRN2", target_bir_lowering=False, debug=False,
                   num_devices=N_CORES)

    x0T_in = nc.dram_tensor("x0T", [D, NL], FP32R, kind="ExternalInput").ap()
    maskT_in = nc.dram_tensor("maskT", [R, N, NL], BF16, kind="ExternalInput").ap()
    wts_in = nc.dram_tensor("wts", [NLAYER, R, D, D], BF16, kind="ExternalInput").ap()
    w12_in = nc.dram_tensor("w12", [NLAYER, R, D, 2], FP32R, kind="ExternalInput").ap()
    lng_in = nc.dram_tensor("lng", [1, D], FP32, kind="ExternalInput").ap()
    lnb_in = nc.dram_tensor("lnb", [1, D], FP32, kind="ExternalInput").ap()

    xenc_out = nc.dram_tensor("xenc", [NL, D], FP32, kind="ExternalOutput").ap()
    xd_out = nc.dram_tensor("xd", [NL, D], FP32, kind="ExternalOutput").ap()

    with ExitStack() as stack:
        tc = stack.enter_context(tile.TileContext(nc))
        ent = stack.enter_context
        cpool = ent(tc.tile_pool(name="const", bufs=1))
        ppool = ent(tc.tile_pool(name="persist", bufs=1))
        wpool = ent(tc.tile_pool(name="w", bufs=2))
        whlpool = ent(tc.tile_pool(name="whl", bufs=4))
        whfpool = ent(tc.tile_pool(name="whf", bufs=6))
        attpool = ent(tc.tile_pool(name="att", bufs=4))
        f1bpool = ent(tc.tile_pool(name="f1b", bufs=8))
        f12pool = ent(tc.tile_pool(name="f12", bufs=8))
        epipool = ent(tc.tile_pool(name="epi", bufs=2))
        xmaxpool = ent(tc.tile_pool(name="xmax", bufs=4))
        smallpool = ent(tc.tile_pool(name="small", bufs=4))
        ps_wh = ent(tc.tile_pool(name="ps_wh", bufs=2, space="PSUM"))
        ps_o = ent(tc.tile_pool(name="ps_o", bufs=2, space="PSUM"))
        ps_s = ent(tc.tile_pool(name="ps_s", bufs=1, space="PSUM"))
        ps_misc = ent(tc.tile_pool(name="ps_misc", bufs=3, space="PSUM"))
        dram_in = ent(tc.tile_pool(name="dram_in", bufs=2, space="DRAM"))
        dram_ag = ent(tc.tile_pool(name="dram_ag", bufs=2, space="DRAM"))
        if True:
            # ---- constants / static inputs
            nonce = os.environ.get("KERNEL_NONCE", "0")
            ident = cpool.tile([128, 128], FP32, name=f"ident_{nonce}",
                               tag="ident")
            masks.make_identity(nc, ident[:])
            ones_f = cpool.tile([128, 1], FP32, name="ones_f", tag="ones_f")
            nc.vector.memset(ones_f[:], 1.0)
            ones_b = cpool.tile([128, 1], BF16, name="ones_b", tag="ones_b")
            nc.vector.tensor_copy(ones_b[:], ones_f[:])

            if not ln_trivial:
                g_row = cpool.tile([1, D], FP32, name="g_row", tag="g_row")
                nc.sync.dma_start(g_row[:], lng_in)
                b_row = cpool.tile([1, D], FP32, name="b_row", tag="b_row")
                nc.sync.dma_start(b_row[:], lnb_in)
                g_bc = cpool.tile([128, D], FP32, name="g_bc", tag="g_bc")
                nc.gpsimd.partition_broadcast(g_bc[:], g_row[:])
                b_bc = cpool.tile([128, D], FP32, name="b_bc", tag="b_bc")
                nc.gpsimd.partition_broadcast(b_bc[:], b_row[:])

            w12_sb = cpool.tile([128, NLAYER, R, KJ, 2], FP32R,
                                name="w12_sb", tag="w12_sb")
            nc.sync.dma_start(
                w12_sb[:], w12_in.rearrange("l r (j p) c -> p l r j c", p=128))

            maskT_sb = cpool.tile([128, R, NT, NL], BF16,
                                  name="maskT_sb", tag="maskT_sb")
            for r in range(R):
                nc.sync.dma_start(
                    maskT_sb[:, r, :, :],
                    maskT_in[r].rearrange("(t p) n -> p t n", p=128))

            # per-layer transposed activations: bf16 for the Wh matmul,
            # fp32r for the f12 logit matmul
            xTb = [ppool.tile([128, KJ, NL], BF16, name=f"xTb{l}", tag=f"xTb{l}")
                   for l in range(NLAYER)]
            xTr = [ppool.tile([128, KJ, NL], FP32R, name=f"xTr{l}", tag=f"xTr{l}")
                   for l in range(NLAYER)]
            nc.sync.dma_start(xTr[0][:], x0T_in.rearrange("(j p) n -> p j n", p=128))
            nc.vector.tensor_copy(xTb[0][:], xTr[0][:])

            for L in range(NLAYER):
                # ---- phase 0: attention logit halves f12T, tiny AllGather
                f12_in_d = dram_in.tile([2 * R, NL], FP32, name="f12_in",
                                        tag="f12_in")
                f12_sb = []
                for r in range(R):
                    f12_ps = ps_misc.tile([2, NL], FP32, name="f12_ps",
                                          tag="misc")
                    for j in range(KJ):
                        nc.tensor.matmul(
                            f12_ps[:], w12_sb[:, L, r, j, :], xTr[L][:, j, :],
                            start=(j == 0), stop=(j == KJ - 1))
                    f12 = f12pool.tile([2, NL], FP32, name="f12", tag="f12")
                    nc.vector.tensor_copy(f12[:], f12_ps[:])
                    f12_sb.append(f12)
                    nc.sync.dma_start(f12_in_d[2 * r:2 * r + 2, :], f12[:])
                f12_ag_d = dram_ag.tile([2 * R * N_CORES, NL], FP32,
                                        addr_space="Shared", name="f12_ag",
                                        tag="f12_ag")
                nc.gpsimd.collective_compute(
                    "AllGather", ALU.bypass, replica_groups=RG,
                    ins=[f12_in_d[:].opt()], outs=[f12_ag_d[:].opt()])

                # ---- phase 1+2: Wh_local (bf16), TWO AllGathers per
                # layer (relations 01 then 23) so phase 3 of the first pair
                # overlaps the second collective
                wh_ag = []
                for g in range(2):
                    wh_in_d = dram_in.tile([2 * NL, D], BF16,
                                           name=f"wh_in{g}", tag=f"wh_in{g}")
                    for rr in range(2):
                        r = 2 * g + rr
                        w_sb = wpool.tile([128, KJ, D], BF16, name="w_sb",
                                          tag="w_sb")
                        nc.sync.dma_start(
                            w_sb[:],
                            wts_in[L, r].rearrange("(j p) o -> p j o", p=128))
                        for h in range(2):
                            wh_ps = ps_wh.tile([128, D], FP32, name="wh_ps",
                                               tag="wh_ps")
                            for j in range(KJ):
                                nc.tensor.matmul(
                                    wh_ps[:],
                                    xTb[L][:, j, h * 128:(h + 1) * 128],
                                    w_sb[:, j, :],
                                    start=(j == 0), stop=(j == KJ - 1))
                            whl = whlpool.tile([128, D], BF16, name="whl",
                                               tag="whl")
                            nc.vector.tensor_copy(whl[:], wh_ps[:])
                            nc.sync.dma_start(
                                wh_in_d[rr * NL + h * 128:
                                        rr * NL + (h + 1) * 128, :],
                                whl[:])
                    wh_ag_d = dram_ag.tile([N_CORES * 2 * NL, D], BF16,
                                           addr_space="Shared",
                                           name=f"wh_ag{g}", tag=f"wh_ag{g}")
                    nc.gpsimd.collective_compute(
                        "AllGather", ALU.bypass, replica_groups=RG,
                        ins=[wh_in_d[:].opt()], outs=[wh_ag_d[:].opt()])
                    wh_ag.append(wh_ag_d)

                # f1 broadcast tiles (local rows), f2 per-partition tiles
                f1_bc = []
                for r in range(R):
                    f1b = f1bpool.tile([128, NL], FP32, name="f1b", tag="f1b")
                    nc.gpsimd.partition_broadcast(f1b[:], f12_sb[r][0:1, :])
                    f1_bc.append(f1b)
                f2_sb = smallpool.tile([128, R, 2, N_CORES], FP32, name="f2",
                                       tag="f2")
                for r in range(R):
                    f2rows = smallpool.tile([N_CORES, NL], FP32, name="f2rows",
                                            tag="f2rows")
                    nc.sync.dma_start(f2rows[:],
                                      f12_ag_d[(2 * r + 1)::(2 * R), :])
                    for h in range(2):
                        f2t_ps = ps_misc.tile([128, N_CORES], FP32,
                                              name="f2t_ps", tag="misc")
                        nc.tensor.transpose(
                            f2t_ps[:], f2rows[:, h * 128:(h + 1) * 128],
                            ident[0:N_CORES, 0:N_CORES])
                        nc.vector.tensor_copy(f2_sb[:, r, h, :], f2t_ps[:])

                # ---- phase 3: attention + epilogue per relation
                xmax = [xmaxpool.tile([128, D], FP32, name="xmax", tag="xmax")
                        for _ in range(2)]
                for r in range(R):
                    o_ps = [ps_o.tile([128, D], FP32, name="o_ps", tag="o_ps")
                            for _ in range(2)]
                    s_ps = ps_s.tile([1, NL], FP32, name="s_ps", tag="s_ps")
                    for tp in range(NT // 2):
                        # pair of m-tiles t=2tp, 2tp+1: contiguous 256 rows
                        # of the gathered Wh and a contiguous 512-wide mask
                        row0 = tp * (2 * NL) + (r % 2) * NL
                        whf = whfpool.tile([128, 2, D], BF16, name="whf",
                                           tag="whf")
                        nc.sync.dma_start(
                            whf[:],
                            wh_ag[r // 2][row0:row0 + 2 * 128, :].rearrange(
                                "(u p) d -> p u d", p=128))
                        z2 = attpool.tile([128, 2, NL], FP32, name="z2",
                                          tag="z2")
                        for hh in range(2):
                            nc.scalar.activation(z2[:, hh, :], f1_bc[r][:],
                                                 AF.Prelu,
                                                 bias=f2_sb[:, r, hh,
                                                            tp:tp + 1],
                                                 alpha=ALPHA)
                        e2 = attpool.tile([128, 2 * NL], BF16, name="e2",
                                          tag="e2")
                        nc.scalar.activation(e2[:], z2[:].rearrange("p u n -> p (u n)"), AF.Exp)
                        pt = attpool.tile([128, 2 * NL], BF16, name="pt",
                                          tag="pt")
                        nc.gpsimd.tensor_mul(
                            pt[:], e2[:],
                            maskT_sb[:, r, 2 * tp:2 * tp + 2, :].rearrange(
                                "p t n -> p (t n)"))
                        for u in range(2):
                            for h in range(2):
                                nc.tensor.matmul(
                                    o_ps[h][:],
                                    pt[:, u * NL + h * 128:
                                       u * NL + (h + 1) * 128],
                                    whf[:, u, :],
                                    start=(tp == 0 and u == 0),
                                    stop=(tp == NT // 2 - 1 and u == 1))
                        for u in range(2):
                            nc.tensor.matmul(
                                s_ps[:], ones_b[:],
                                pt[:, u * NL:(u + 1) * NL],
                                start=(tp == 0 and u == 0),
                                stop=(tp == NT // 2 - 1 and u == 1))

                    rs_row = smallpool.tile([1, NL], FP32, name="rs_row",
                                            tag="rs_row")
                    nc.vector.reciprocal(rs_row[:], s_ps[:])
                    for h in range(2):
                        rs_ps = ps_misc.tile([128, 1], FP32, name="rs_ps",
                                             tag="misc")
                        nc.tensor.transpose(rs_ps[:],
                                            rs_row[:, h * 128:(h + 1) * 128],
                                            ident[0:1, 0:1])
                        rs_col = smallpool.tile([128, 1], FP32, name="rs_col",
                                                tag="rs_col")
                        nc.vector.tensor_copy(rs_col[:], rs_ps[:])

                        xo = epipool.tile([128, D], FP32, name="xo", tag="xo")
                        nc.vector.tensor_scalar_mul(xo[:], o_ps[h][:],
                                                    rs_col[:, 0:1])
                        # elu(xo) + 1 = relu(xo) + exp(min(xo, 0))
                        sum1 = smallpool.tile([128, 1], FP32, name="sum1",
                                              tag="sum1")
                        r1 = epipool.tile([128, D], FP32, name="r1", tag="r1")
                        nc.vector.tensor_scalar(r1[:], xo[:], 0.0, 0.0,
                                                op0=ALU.max, op1=ALU.add,
                                                accum_out=sum1[:])
                        xm = epipool.tile([128, D], FP32, name="xm", tag="xm")
                        nc.vector.tensor_scalar_min(xm[:], xo[:], 0.0)
                        sum2 = smallpool.tile([128, 1], FP32, name="sum2",
                                              tag="sum2")
                        ce = epipool.tile([128, D], FP32, name="ce", tag="ce")
                        nc.scalar.activation(ce[:], xm[:], AF.Exp,
                                             accum_out=sum2[:])
                        xe = epipool.tile([128, D], FP32, name="xe", tag="xe")
                        nc.vector.tensor_add(xe[:], r1[:], ce[:])

                        if L < NLAYER - 1:
                            # layernorm over d (shift-invariant: xe = elu + 1)
                            musum = smallpool.tile([128, 1], FP32, name="musum",
                                                   tag="musum")
                            nc.vector.tensor_add(musum[:], sum1[:], sum2[:])
                            negmu = smallpool.tile([128, 1], FP32, name="negmu",
                                                   tag="negmu")
                            nc.vector.tensor_scalar_mul(negmu[:], musum[:],
                                                        -1.0 / D)
                            ctr = epipool.tile([128, D], FP32, name="ctr",
                                               tag="ctr")
                            nc.vector.tensor_scalar_add(ctr[:], xe[:],
                                                        negmu[:, 0:1])
                            ss = smallpool.tile([128, 1], FP32, name="ss",
                                                tag="ss")
                            sq = epipool.tile([128, D], FP32, name="sq",
                                              tag="sq")
                            nc.vector.scalar_tensor_tensor(
                                sq[:], ctr[:], 1.0, ctr[:],
                                op0=ALU.mult, op1=ALU.mult, accum_out=ss[:])
                            vt = smallpool.tile([128, 1], FP32, name="vt",
                                                tag="vt")
                            nc.vector.tensor_scalar(vt[:], ss[:], 1.0 / D, EPS,
                                                    op0=ALU.mult, op1=ALU.add)
                            lnv = smallpool.tile([128, 1], FP32, name="lnv",
                                                 tag="lnv")
                            nc.scalar.activation(lnv[:], vt[:], AF.Ln)
                            rstd = smallpool.tile([128, 1], FP32, name="rstd",
                                                  tag="rstd")
                            nc.scalar.activation(rstd[:], lnv[:], AF.Exp,
                                                 scale=-0.5)
                            if ln_trivial:
                                if r == 0:
                                    nc.vector.tensor_scalar_mul(
                                        xmax[h][:], ctr[:], rstd[:, 0:1])
                                else:
                                    yv = epipool.tile([128, D], FP32,
                                                      name="yv", tag="yv")
                                    nc.vector.tensor_scalar_mul(
                                        yv[:], ctr[:], rstd[:, 0:1])
                                    nc.vector.tensor_max(xmax[h][:],
                                                         xmax[h][:], yv[:])
                            else:
                                yg = epipool.tile([128, D], FP32, name="yg",
                                                  tag="yg")
                                nc.vector.scalar_tensor_tensor(
                                    yg[:], ctr[:], rstd[:, 0:1], g_bc[:],
                                    op0=ALU.mult, op1=ALU.mult)
                                if r == 0:
                                    nc.vector.tensor_add(xmax[h][:], yg[:],
                                                         b_bc[:])
                                else:
                                    yb = epipool.tile([128, D], FP32,
                                                      name="yb", tag="yb")
                                    nc.vector.tensor_add(yb[:], yg[:], b_bc[:])
                                    nc.vector.tensor_max(xmax[h][:],
                                                         xmax[h][:], yb[:])
                        else:
                            if r == 0:
                                nc.vector.tensor_scalar_add(xmax[h][:], xe[:],
                                                            -1.0)
                            else:
                                ym = epipool.tile([128, D], FP32, name="ym",
                                                  tag="ym")
                                nc.vector.tensor_scalar_add(ym[:], xe[:], -1.0)
                                nc.vector.tensor_max(xmax[h][:], xmax[h][:],
                                                     ym[:])

                # ---- phase 4: outputs / transpose for next layer
                if L == 1:
                    for h in range(2):
                        nc.sync.dma_start(xenc_out[h * 128:(h + 1) * 128, :],
                                          xmax[h][:])
                if L == NLAYER - 1:
                    for h in range(2):
                        nc.sync.dma_start(xd_out[h * 128:(h + 1) * 128, :],
                                          xmax[h][:])
                else:
                    for h in range(2):
                        for j in range(KJ):
                            tr_ps = ps_misc.tile([128, 128], FP32,
                                                 name="tr_ps", tag="misc")
                            nc.tensor.transpose(
                                tr_ps[:], xmax[h][:, j * 128:(j + 1) * 128],
                                ident[:])
                            nc.vector.tensor_copy(
                                xTr[L + 1][:, j, h * 128:(h + 1) * 128],
                                tr_ps[:])
                            nc.vector.tensor_copy(
                                xTb[L + 1][:, j, h * 128:(h + 1) * 128],
                                tr_ps[:])

    nc.compile()
    return nc


def _get_program(ln_trivial):
    if ln_trivial not in _PROGRAMS:
        _PROGRAMS[ln_trivial] = _build_program(ln_trivial)
    return _PROGRAMS[ln_trivial]


def _softmax(x, axis=-1):
    m = x.max(axis=axis, keepdims=True)
    e = np.exp(x - m)
    return e / e.sum(axis=axis, keepdims=True)


def _run_device(x0, rel_adj, W_all, w12, ln_g, ln_b, trace=False):
    import ml_dtypes
    from concourse import bass_utils

    ln_g = np.ascontiguousarray(ln_g.reshape(1, D), dtype=np.float32)
    ln_b = np.ascontiguousarray(ln_b.reshape(1, D), dtype=np.float32)
    ln_trivial = bool(np.all(ln_g == 1.0) and np.all(ln_b == 0.0))
    nc = _get_program(ln_trivial)

    maskT = (rel_adj > 0).transpose(0, 2, 1)  # [R, m, n]
    wts = np.ascontiguousarray(W_all).astype(ml_dtypes.bfloat16)
    w12 = np.ascontiguousarray(w12, dtype=np.float32)

    in_maps = []
    for c in range(N_CORES):
        sl = slice(c * NL, (c + 1) * NL)
        in_maps.append({
            "x0T": np.ascontiguousarray(x0[sl].T, dtype=np.float32),
            "maskT": np.ascontiguousarray(maskT[:, :, sl]).astype(
                ml_dtypes.bfloat16),
            "wts": wts,
            "w12": w12,
            "lng": ln_g,
            "lnb": ln_b,
        })
    res = bass_utils.run_bass_kernel_spmd(
        nc, in_maps, core_ids=list(range(N_CORES)), trace=trace)
    xenc = np.concatenate([res.results[c]["xenc"] for c in range(N_CORES)], 0)
    xd = np.concatenate([res.results[c]["xd"] for c in range(N_CORES)], 0)
    return xenc, xd, res


def kernel(x, rel_adj, qembedding1, qnode_idx, choices_nodes_idx,
           Wc, bc, wq, enc_W, enc_a1, enc_a2, dec_W, dec_a1, dec_a2,
           ln_g, ln_b, Wqc, bqc, Wout, bout, _trace=False):
    x = np.asarray(x, dtype=np.float32)
    qembedding1 = np.asarray(qembedding1, dtype=np.float32)

    # QuestionLayer + question-gated node embeddings (host, tiny)
    qemb = _softmax(qembedding1 @ np.asarray(wq, np.float32))[None, :] @ qembedding1
    qemb = qemb[0]
    v = np.asarray(Wc, np.float32).T @ qemb
    beta = np.asarray(bc, np.float32) @ qemb
    p = 1.0 / (1.0 + np.exp(-(x @ v + beta)))[:, None]
    x0 = p * x + (1.0 - p) * qemb[None, :]

    # stack per-layer weights: enc h=0,1 then dec i=0,1
    W_all = np.stack([enc_W[0], enc_W[1], dec_W[0], dec_W[1]], 0)
    a1_all = np.stack([enc_a1[0], enc_a1[1], dec_a1[0], dec_a1[1]], 0)
    a2_all = np.stack([enc_a2[0], enc_a2[1], dec_a2[0], dec_a2[1]], 0)
    w1 = np.einsum("lrio,lro->lri", W_all, a1_all)
    w2 = np.einsum("lrio,lro->lri", W_all, a2_all)
    w12 = np.stack([w1, w2], axis=-1)  # [L, R, D, 2]

    xenc, xd, res = _run_device(x0, np.asarray(rel_adj), W_all, w12,
                                np.asarray(ln_g), np.asarray(ln_b),
                                trace=_trace)

    # QuestionChoiceLayer (host, tiny)
    cn = xenc[np.asarray(choices_nodes_idx)]              # [5, C, D]
    qproj = qembedding1.mean(0) @ np.asarray(Wqc, np.float32) + np.asarray(bqc, np.float32)
    sc = _softmax(cn @ qproj, axis=-1)                    # [5, C]
    cembed = np.einsum("kc,kcd->kd", sc, cn)              # [5, D]
    logit = cembed @ np.asarray(Wout, np.float32) + np.asarray(bout, np.float32)
    m = logit.max(axis=0, keepdims=True)
    cout = logit - (m + np.log(np.exp(logit - m).sum(axis=0, keepdims=True)))

    if _trace:
        return (cout, xd), res
    return cout, xd


# revision 16
# speedup vs baseline: 1.0211x; 1.0211x over previous
"""Multi-relational GAT (2-layer encoder + 2-layer decoder) on 8 TRN2 NeuronCores.

Row-parallel over the N=2048 nodes (256 rows/core). Per GAT layer:
  1. each core computes Wh_local = x_local @ W[r] (bf16 matmuls, fp32 psum)
     and f12T_local = [W@a1 | W@a2]^T @ x_local (fp32r, attention logits),
  2. AllGather f12T (tiny, fp32) and Wh (bf16, one collective for all 4
     relations) across the 8 cores,
  3. each core builds its transposed attention tile PT[m, n] =
     exp(leakyrelu(f1[n] + f2[m])) * adjT[m, n] (ACT Prelu+Exp, GpSimd mask
     multiply) and accumulates out[n, d] = sum_m PT[m, n] * Wh[m, d] plus
     row sums s[n] on the PE,
  4. normalize by 1/s, ELU, (LayerNorm), maxpool over the 4 relations.
The question-gating front end and the tiny 5-choice head run on host numpy.
"""
import os
import numpy as np

N_CORES = 8
N, D, R, NLAYER = 2048, 512, 4, 4
NL = N // N_CORES          # 256 local rows per core
NT = N // 128              # 16 m-tiles of 128
KJ = D // 128              # 4 k-tiles over the feature dim
ALPHA = 0.2
EPS = 1e-5

_PROGRAMS = {}


def _build_program(ln_trivial):
    import concourse.bacc as bacc
    import concourse.mybir as mybir
    import concourse.tile as tile
    from concourse import masks
    from contextlib import ExitStack

    FP32 = mybir.dt.float32
    FP32R = mybir.dt.float32r
    BF16 = mybir.dt.bfloat16
    AF = mybir.ActivationFunctionType
    ALU = mybir.AluOpType
    RG = [list(range(N_CORES))]

    nc = bacc.Bacc("TRN2", target_bir_lowering=False, debug=False,
                   num_devices=N_CORES)

    x0T_in = nc.dram_tensor("x0T", [D, NL], FP32R, kind="ExternalInput").ap()
    maskT_in = nc.dram_tensor("maskT", [R, N, NL], BF16, kind="ExternalInput").ap()
    wts_in = nc.dram_tensor("wts", [NLAYER, R, D, D], BF16, kind="ExternalInput").ap()
    w12_in = nc.dram_tensor("w12", [NLAYER, R, D, 2], FP32R, kind="ExternalInput").ap()
    lng_in = nc.dram_tensor("lng", [1, D], FP32, kind="ExternalInput").ap()
    lnb_in = nc.dram_tensor("lnb", [1, D], FP32, kind="ExternalInput").ap()

    xenc_out = nc.dram_tensor("xenc", [NL, D], FP32, kind="ExternalOutput").ap()
    xd_out = nc.dram_tensor("xd", [NL, D], FP32, kind="ExternalOutput").ap()

    with ExitStack() as stack:
        tc = stack.enter_context(tile.TileContext(nc))
        ent = stack.enter_context
        cpool = ent(tc.tile_pool(name="const", bufs=1))
        ppool = ent(tc.tile_pool(name="persist", bufs=1))
        wpool = ent(tc.tile_pool(name="w", bufs=3))
        whlpool = ent(tc.tile_pool(name="whl", bufs=6))
        whfpool = ent(tc.tile_pool(name="whf", bufs=6))
        attpool = ent(tc.tile_pool(name="att", bufs=6))
        f1bpool = ent(tc.tile_pool(name="f1b", bufs=8))
        f12pool = ent(tc.tile_pool(name="f12", bufs=8))
        epipool = ent(tc.tile_pool(name="epi", bufs=3))
        xmaxpool = ent(tc.tile_pool(name="xmax", bufs=4))
        smallpool = ent(tc.tile_pool(name="small", bufs=4))
        ps_wh = ent(tc.tile_pool(name="ps_wh", bufs=2, space="PSUM"))
        ps_o = ent(tc.tile_pool(name="ps_o", bufs=2, space="PSUM"))
        ps_s = ent(tc.tile_pool(name="ps_s", bufs=1, space="PSUM"))
        ps_misc = ent(tc.tile_pool(name="ps_misc", bufs=3, space="PSUM"))
        dram_in = ent(tc.tile_pool(name="dram_in", bufs=2, space="DRAM"))
        dram_ag = ent(tc.tile_pool(name="dram_ag", bufs=2, space="DRAM"))
        if True:
            # ---- constants / static inputs
            nonce = os.environ.get("KERNEL_NONCE", "0")
            ident = cpool.tile([128, 128], FP32, name=f"ident_{nonce}",
                               tag="ident")
            masks.make_identity(nc, ident[:])
            ones_f = cpool.tile([128, 1], FP32, name="ones_f", tag="ones_f")
            nc.vector.memset(ones_f[:], 1.0)
            ones_b = cpool.tile([128, 1], BF16, name="ones_b", tag="ones_b")
            nc.vector.tensor_copy(ones_b[:], ones_f[:])

            if not ln_trivial:
                g_row = cpool.tile([1, D], FP32, name="g_row", tag="g_row")
                nc.sync.dma_start(g_row[:], lng_in)
                b_row = cpool.tile([1, D], FP32, name="b_row", tag="b_row")
                nc.sync.dma_start(b_row[:], lnb_in)
                g_bc = cpool.tile([128, D], FP32, name="g_bc", tag="g_bc")
                nc.gpsimd.partition_broadcast(g_bc[:], g_row[:])
                b_bc = cpool.tile([128, D], FP32, name="b_bc", tag="b_bc")
                nc.gpsimd.partition_broadcast(b_bc[:], b_row[:])

            w12_sb = cpool.tile([128, NLAYER, R, KJ, 2], FP32R,
                                name="w12_sb", tag="w12_sb")
            nc.sync.dma_start(
                w12_sb[:], w12_in.rearrange("l r (j p) c -> p l r j c", p=128))

            maskT_sb = cpool.tile([128, R, NT, NL], BF16,
                                  name="maskT_sb", tag="maskT_sb")
            for r in range(R):
                nc.sync.dma_start(
                    maskT_sb[:, r, :, :],
                    maskT_in[r].rearrange("(t p) n -> p t n", p=128))

            # per-layer transposed activations: bf16 for the Wh matmul,
            # fp32r for the f12 logit matmul
            xTb = [ppool.tile([128, KJ, NL], BF16, name=f"xTb{l}", tag=f"xTb{l}")
                   for l in range(NLAYER)]
            xTr = [ppool.tile([128, KJ, NL], FP32R, name=f"xTr{l}", tag=f"xTr{l}")
                   for l in range(NLAYER)]
            nc.sync.dma_start(xTr[0][:], x0T_in.rearrange("(j p) n -> p j n", p=128))
            nc.vector.tensor_copy(xTb[0][:], xTr[0][:])

            for L in range(NLAYER):
                # ---- phase 0: attention logit halves f12T, tiny AllGather
                f12_in_d = dram_in.tile([2 * R, NL], FP32, name="f12_in",
                                        tag="f12_in")
                f12_sb = []
                for r in range(R):
                    f12_ps = ps_misc.tile([2, NL], FP32, name="f12_ps",
                                          tag="misc")
                    for j in range(KJ):
                        nc.tensor.matmul(
                            f12_ps[:], w12_sb[:, L, r, j, :], xTr[L][:, j, :],
                            start=(j == 0), stop=(j == KJ - 1))
                    f12 = f12pool.tile([2, NL], FP32, name="f12", tag="f12")
                    nc.vector.tensor_copy(f12[:], f12_ps[:])
                    f12_sb.append(f12)
                    nc.sync.dma_start(f12_in_d[2 * r:2 * r + 2, :], f12[:])
                f12_ag_d = dram_ag.tile([2 * R * N_CORES, NL], FP32,
                                        addr_space="Shared", name="f12_ag",
                                        tag="f12_ag")
                nc.gpsimd.collective_compute(
                    "AllGather", ALU.bypass, replica_groups=RG,
                    ins=[f12_in_d[:].opt()], outs=[f12_ag_d[:].opt()])

                # ---- phase 1+2: Wh_local (bf16), TWO AllGathers per
                # layer (relations 01 then 23) so phase 3 of the first pair
                # overlaps the second collective
                wh_ag = []
                for g in range(2):
                    wh_in_d = dram_in.tile([2 * NL, D], BF16,
                                           name=f"wh_in{g}", tag=f"wh_in{g}")
                    for rr in range(2):
                        r = 2 * g + rr
                        w_sb = wpool.tile([128, KJ, D], BF16, name="w_sb",
                                          tag="w_sb")
                        nc.sync.dma_start(
                            w_sb[:],
                            wts_in[L, r].rearrange("(j p) o -> p j o", p=128))
                        for h in range(2):
                            wh_ps = ps_wh.tile([128, D], FP32, name="wh_ps",
                                               tag="wh_ps")
                            for j in range(KJ):
                                nc.tensor.matmul(
                                    wh_ps[:],
                                    xTb[L][:, j, h * 128:(h + 1) * 128],
                                    w_sb[:, j, :],
                                    start=(j == 0), stop=(j == KJ - 1))
                            whl = whlpool.tile([128, D], BF16, name="whl",
                                               tag="whl")
                            nc.vector.tensor_copy(whl[:], wh_ps[:])
                            nc.sync.dma_start(
                                wh_in_d[rr * NL + h * 128:
                                        rr * NL + (h + 1) * 128, :],
                                whl[:])
                    wh_ag_d = dram_ag.tile([N_CORES * 2 * NL, D], BF16,
                                           addr_space="Shared",
                                           name=f"wh_ag{g}", tag=f"wh_ag{g}")
                    nc.gpsimd.collective_compute(
                        "AllGather", ALU.bypass, replica_groups=RG,
                        ins=[wh_in_d[:].opt()], outs=[wh_ag_d[:].opt()])
                    wh_ag.append(wh_ag_d)

                # f1 broadcast tiles (local rows), f2 per-partition tiles
                f1_bc = []
                for r in range(R):
                    f1b = f1bpool.tile([128, NL], FP32, name="f1b", tag="f1b")
                    nc.gpsimd.partition_broadcast(f1b[:], f12_sb[r][0:1, :])
                    f1_bc.append(f1b)
                f2_sb = smallpool.tile([128, R, 2, N_CORES], FP32, name="f2",
                                       tag="f2")
                for r in range(R):
                    f2rows = smallpool.tile([N_CORES, NL], FP32, name="f2rows",
                                            tag="f2rows")
                    nc.sync.dma_start(f2rows[:],
                                      f12_ag_d[(2 * r + 1)::(2 * R), :])
                    for h in range(2):
                        f2t_ps = ps_misc.tile([128, N_CORES], FP32,
                                              name="f2t_ps", tag="misc")
                        nc.tensor.transpose(
                            f2t_ps[:], f2rows[:, h * 128:(h + 1) * 128],
                            ident[0:N_CORES, 0:N_CORES])
                        nc.vector.tensor_copy(f2_sb[:, r, h, :], f2t_ps[:])

                # ---- phase 3: attention + epilogue per relation
                xmax = [xmaxpool.tile([128, D], FP32, name="xmax", tag="xmax")
                        for _ in range(2)]
                for r in range(R):
                    o_ps = [ps_o.tile([128, D], FP32, name="o_ps", tag="o_ps")
                            for _ in range(2)]
                    s_ps = ps_s.tile([1, NL], FP32, name="s_ps", tag="s_ps")
                    for tp in range(NT // 2):
                        # pair of m-tiles t=2tp, 2tp+1: contiguous 256 rows
                        # of the gathered Wh and a contiguous 512-wide mask
                        row0 = tp * (2 * NL) + (r % 2) * NL
                        whf = whfpool.tile([128, 2, D], BF16, name="whf",
                                           tag="whf")
                        nc.sync.dma_start(
                            whf[:],
                            wh_ag[r // 2][row0:row0 + 2 * 128, :].rearrange(
                                "(u p) d -> p u d", p=128))
                        z2 = attpool.tile([128, 2, NL], FP32, name="z2",
                                          tag="z2")
                        for hh in range(2):
                            nc.scalar.activation(z2[:, hh, :], f1_bc[r][:],
                                                 AF.Prelu,
                                                 bias=f2_sb[:, r, hh,
                                                            tp:tp + 1],
                                                 alpha=ALPHA)
                        e2 = attpool.tile([128, 2 * NL], BF16, name="e2",
                                          tag="e2")
                        nc.scalar.activation(e2[:], z2[:].rearrange("p u n -> p (u n)"), AF.Exp)
                        pt = attpool.tile([128, 2 * NL], BF16, name="pt",
                                          tag="pt")
                        nc.gpsimd.tensor_mul(
                            pt[:], e2[:],
                            maskT_sb[:, r, 2 * tp:2 * tp + 2, :].rearrange(
                                "p t n -> p (t n)"))
                        for u in range(2):
                            for h in range(2):
                                nc.tensor.matmul(
                                    o_ps[h][:],
                                    pt[:, u * NL + h * 128:
                                       u * NL + (h + 1) * 128],
                                    whf[:, u, :],
                                    start=(tp == 0 and u == 0),
                                    stop=(tp == NT // 2 - 1 and u == 1))
                        for u in range(2):
                            nc.tensor.matmul(
                                s_ps[:], ones_b[:],
                                pt[:, u * NL:(u + 1) * NL],
                                start=(tp == 0 and u == 0),
                                stop=(tp == NT // 2 - 1 and u == 1))

                    rs_row = smallpool.tile([1, NL], FP32, name="rs_row",
                                            tag="rs_row")
                    nc.vector.reciprocal(rs_row[:], s_ps[:])
                    for h in range(2):
                        rs_ps = ps_misc.tile([128, 1], FP32, name="rs_ps",
                                             tag="misc")
                        nc.tensor.transpose(rs_ps[:],
                                            rs_row[:, h * 128:(h + 1) * 128],
                                            ident[0:1, 0:1])
                        rs_col = smallpool.tile([128, 1], FP32, name="rs_col",
                                                tag="rs_col")
                        nc.vector.tensor_copy(rs_col[:], rs_ps[:])

                        xo = epipool.tile([128, D], FP32, name="xo", tag="xo")
                        nc.vector.tensor_scalar_mul(xo[:], o_ps[h][:],
                                                    rs_col[:, 0:1])
                        # elu(xo) + 1 = relu(xo) + exp(min(xo, 0))
                        sum1 = smallpool.tile([128, 1], FP32, name="sum1",
                                              tag="sum1")
                        r1 = epipool.tile([128, D], FP32, name="r1", tag="r1")
                        nc.vector.tensor_scalar(r1[:], xo[:], 0.0, 0.0,
                                                op0=ALU.max, op1=ALU.add,
                                                accum_out=sum1[:])
                        xm = epipool.tile([128, D], FP32, name="xm", tag="xm")
                        nc.vector.tensor_scalar_min(xm[:], xo[:], 0.0)
                        sum2 = smallpool.tile([128, 1], FP32, name="sum2",
                                              tag="sum2")
                        ce = epipool.tile([128, D], FP32, name="ce", tag="ce")
                        nc.scalar.activation(ce[:], xm[:], AF.Exp,
                                             accum_out=sum2[:])
                        xe = epipool.tile([128, D], FP32, name="xe", tag="xe")
                        nc.vector.tensor_add(xe[:], r1[:], ce[:])

                        if L < NLAYER - 1:
                            # layernorm over d (shift-invariant: xe = elu + 1)
                            musum = smallpool.tile([128, 1], FP32, name="musum",
                                                   tag="musum")
                            nc.vector.tensor_add(musum[:], sum1[:], sum2[:])
                            negmu = smallpool.tile([128, 1], FP32, name="negmu",
                                                   tag="negmu")
                            nc.vector.tensor_scalar_mul(negmu[:], musum[:],
                                                        -1.0 / D)
                            ctr = epipool.tile([128, D], FP32, name="ctr",
                                               tag="ctr")
                            nc.vector.tensor_scalar_add(ctr[:], xe[:],
                                                        negmu[:, 0:1])
                            ss = smallpool.tile([128, 1], FP32, name="ss",
                                                tag="ss")
                            sq = epipool.tile([128, D], FP32, name="sq",
                                              tag="sq")
                            nc.vector.scalar_tensor_tensor(
                                sq[:], ctr[:], 1.0, ctr[:],
                                op0=ALU.mult, op1=ALU.mult, accum_out=ss[:])
                            vt = smallpool.tile([128, 1], FP32, name="vt",
                                                tag="vt")
                            nc.vector.tensor_scalar(vt[:], ss[:], 1.0 / D, EPS,
                                                    op0=ALU.mult, op1=ALU.add)
                            lnv = smallpool.tile([128, 1], FP32, name="lnv",
                                                 tag="lnv")
                            nc.scalar.activation(lnv[:], vt[:], AF.Ln)
                            rstd = smallpool.tile([128, 1], FP32, name="rstd",
                                                  tag="rstd")
                            nc.scalar.activation(rstd[:], lnv[:], AF.Exp,
                                                 scale=-0.5)
                            if ln_trivial:
                                if r == 0:
                                    nc.vector.tensor_scalar_mul(
                                        xmax[h][:], ctr[:], rstd[:, 0:1])
                                else:
                                    yv = epipool.tile([128, D], FP32,
                                                      name="yv", tag="yv")
                                    nc.vector.tensor_scalar_mul(
                                        yv[:], ctr[:], rstd[:, 0:1])
                                    nc.vector.tensor_max(xmax[h][:],
                                                         xmax[h][:], yv[:])
                            else:
                                yg = epipool.tile([128, D], FP32, name="yg",
                                                  tag="yg")
                                nc.vector.scalar_tensor_tensor(
                                    yg[:], ctr[:], rstd[:, 0:1], g_bc[:],
                                    op0=ALU.mult, op1=ALU.mult)
                                if r == 0:
                                    nc.vector.tensor_add(xmax[h][:], yg[:],
                                                         b_bc[:])
                                else:
                                    yb = epipool.tile([128, D], FP32,
                                                      name="yb", tag="yb")
                                    nc.vector.tensor_add(yb[:], yg[:], b_bc[:])
                                    nc.vector.tensor_max(xmax[h][:],
                                                         xmax[h][:], yb[:])
                        else:
                            if r == 0:
                                nc.vector.tensor_scalar_add(xmax[h][:], xe[:],
                                                            -1.0)
                            else:
                                ym = epipool.tile([128, D], FP32, name="ym",
                                                  tag="ym")
                                nc.vector.tensor_scalar_add(ym[:], xe[:], -1.0)
                                nc.vector.tensor_max(xmax[h][:], xmax[h][:],
                                                     ym[:])

                # ---- phase 4: outputs / transpose for next layer
                if L == 1:
                    for h in range(2):
                        nc.sync.dma_start(xenc_out[h * 128:(h + 1) * 128, :],
                                          xmax[h][:])
                if L == NLAYER - 1:
                    for h in range(2):
                        nc.sync.dma_start(xd_out[h * 128:(h + 1) * 128, :],
                                          xmax[h][:])
                else:
                    for h in range(2):
                        for j in range(KJ):
                            tr_ps = ps_misc.tile([128, 128], FP32,
                                                 name="tr_ps", tag="misc")
                            nc.tensor.transpose(
                                tr_ps[:], xmax[h][:, j * 128:(j + 1) * 128],
                                ident[:])
                            nc.vector.tensor_copy(
                                xTr[L + 1][:, j, h * 128:(h + 1) * 128],
                                tr_ps[:])
                            nc.vector.tensor_copy(
                                xTb[L + 1][:, j, h * 128:(h + 1) * 128],
                                tr_ps[:])

    nc.compile()
    return nc


def _get_program(ln_trivial):
    if ln_trivial not in _PROGRAMS:
        _PROGRAMS[ln_trivial] = _build_program(ln_trivial)
    return _PROGRAMS[ln_trivial]


def _softmax(x, axis=-1):
    m = x.max(axis=axis, keepdims=True)
    e = np.exp(x - m)
    return e / e.sum(axis=axis, keepdims=True)


def _run_device(x0, rel_adj, W_all, w12, ln_g, ln_b, trace=False):
    import ml_dtypes
    from concourse import bass_utils

    ln_g = np.ascontiguousarray(ln_g.reshape(1, D), dtype=np.float32)
    ln_b = np.ascontiguousarray(ln_b.reshape(1, D), dtype=np.float32)
    ln_trivial = bool(np.all(ln_g == 1.0) and np.all(ln_b == 0.0))
    nc = _get_program(ln_trivial)

    maskT = (rel_adj > 0).transpose(0, 2, 1)  # [R, m, n]
    wts = np.ascontiguousarray(W_all).astype(ml_dtypes.bfloat16)
    w12 = np.ascontiguousarray(w12, dtype=np.float32)

    in_maps = []
    for c in range(N_CORES):
        sl = slice(c * NL, (c + 1) * NL)
        in_maps.append({
            "x0T": np.ascontiguousarray(x0[sl].T, dtype=np.float32),
            "maskT": np.ascontiguousarray(maskT[:, :, sl]).astype(
                ml_dtypes.bfloat16),
            "wts": wts,
            "w12": w12,
            "lng": ln_g,
            "lnb": ln_b,
        })
    res = bass_utils.run_bass_kernel_spmd(
        nc, in_maps, core_ids=list(range(N_CORES)), trace=trace)
    xenc = np.concatenate([res.results[c]["xenc"] for c in range(N_CORES)], 0)
    xd = np.concatenate([res.results[c]["xd"] for c in range(N_CORES)], 0)
    return xenc, xd, res


def kernel(x, rel_adj, qembedding1, qnode_idx, choices_nodes_idx,
           Wc, bc, wq, enc_W, enc_a1, enc_a2, dec_W, dec_a1, dec_a2,
           ln_g, ln_b, Wqc, bqc, Wout, bout, _trace=False):
    x = np.asarray(x, dtype=np.float32)
    qembedding1 = np.asarray(qembedding1, dtype=np.float32)

    # QuestionLayer + question-gated node embeddings (host, tiny)
    qemb = _softmax(qembedding1 @ np.asarray(wq, np.float32))[None, :] @ qembedding1
    qemb = qemb[0]
    v = np.asarray(Wc, np.float32).T @ qemb
    beta = np.asarray(bc, np.float32) @ qemb
    p = 1.0 / (1.0 + np.exp(-(x @ v + beta)))[:, None]
    x0 = p * x + (1.0 - p) * qemb[None, :]

    # stack per-layer weights: enc h=0,1 then dec i=0,1
    W_all = np.stack([enc_W[0], enc_W[1], dec_W[0], dec_W[1]], 0)
    a1_all = np.stack([enc_a1[0], enc_a1[1], dec_a1[0], dec_a1[1]], 0)
    a2_all = np.stack([enc_a2[0], enc_a2[1], dec_a2[0], dec_a2[1]], 0)
    w1 = np.einsum("lrio,lro->lri", W_all, a1_all)
    w2 = np.einsum("lrio,lro->lri", W_all, a2_all)
    w12 = np.stack([w1, w2], axis=-1)  # [L, R, D, 2]

    xenc, xd, res = _run_device(x0, np.asarray(rel_adj), W_all, w12,
                                np.asarray(ln_g), np.asarray(ln_b),
                                trace=_trace)

    # QuestionChoiceLayer (host, tiny)
    cn = xenc[np.asarray(choices_nodes_idx)]              # [5, C, D]
    qproj = qembedding1.mean(0) @ np.asarray(Wqc, np.float32) + np.asarray(bqc, np.float32)
    sc = _softmax(cn @ qproj, axis=-1)                    # [5, C]
    cembed = np.einsum("kc,kcd->kd", sc, cn)              # [5, D]
    logit = cembed @ np.asarray(Wout, np.float32) + np.asarray(bout, np.float32)
    m = logit.max(axis=0, keepdims=True)
    cout = logit - (m + np.log(np.exp(logit - m).sum(axis=0, keepdims=True)))

    if _trace:
        return (cout, xd), res
    return cout, xd


# revision 21
# speedup vs baseline: 1.0311x; 1.0099x over previous
"""Multi-relational GAT (2-layer encoder + 2-layer decoder) on 8 TRN2 NeuronCores.

Row-parallel over the N=2048 nodes (256 rows/core). Per GAT layer:
  1. each core computes Wh_local = x_local @ W[r] (bf16 matmuls, fp32 psum)
     and f12T_local = [W@a1 | W@a2]^T @ x_local (fp32r, attention logits),
  2. AllGather f12T (tiny, fp32) and Wh (bf16, one collective for all 4
     relations) across the 8 cores,
  3. each core builds its transposed attention tile PT[m, n] =
     exp(leakyrelu(f1[n] + f2[m])) * adjT[m, n] (ACT Prelu+Exp, GpSimd mask
     multiply) and accumulates out[n, d] = sum_m PT[m, n] * Wh[m, d] plus
     row sums s[n] on the PE,
  4. normalize by 1/s, ELU, (LayerNorm), maxpool over the 4 relations.
The question-gating front end and the tiny 5-choice head run on host numpy.
"""
import os
import numpy as np

N_CORES = 8
N, D, R, NLAYER = 2048, 512, 4, 4
NL = N // N_CORES          # 256 local rows per core
NT = N // 128              # 16 m-tiles of 128
KJ = D // 128              # 4 k-tiles over the feature dim
ALPHA = 0.2
EPS = 1e-5

_PROGRAMS = {}


def _build_program(ln_trivial):
    import concourse.bacc as bacc
    import concourse.mybir as mybir
    import concourse.tile as tile
    from concourse import masks
    from contextlib import ExitStack

    FP32 = mybir.dt.float32
    FP32R = mybir.dt.float32r
    BF16 = mybir.dt.bfloat16
    AF = mybir.ActivationFunctionType
    ALU = mybir.AluOpType
    RG = [list(range(N_CORES))]

    nc = bacc.Bacc("TRN2", target_bir_lowering=False, debug=False,
                   num_devices=N_CORES)

    x0T_in = nc.dram_tensor("x0T", [D, NL], FP32R, kind="ExternalInput").ap()
    maskT_in = nc.dram_tensor("maskT", [R, N, NL], BF16, kind="ExternalInput").ap()
    wts_in = nc.dram_tensor("wts", [NLAYER, R, D, D], BF16, kind="ExternalInput").ap()
    w12_in = nc.dram_tensor("w12", [NLAYER, R, D, 2], FP32R, kind="ExternalInput").ap()
    lng_in = nc.dram_tensor("lng", [1, D], FP32, kind="ExternalInput").ap()
    lnb_in = nc.dram_tensor("lnb", [1, D], FP32, kind="ExternalInput").ap()

    xenc_out = nc.dram_tensor("xenc", [NL, D], FP32, kind="ExternalOutput").ap()
    xd_out = nc.dram_tensor("xd", [NL, D], FP32, kind="ExternalOutput").ap()

    with ExitStack() as stack:
        tc = stack.enter_context(tile.TileContext(nc))
        ent = stack.enter_context
        cpool = ent(tc.tile_pool(name="const", bufs=1))
        ppool = ent(tc.tile_pool(name="persist", bufs=1))
        wpool = ent(tc.tile_pool(name="w", bufs=3))
        whlpool = ent(tc.tile_pool(name="whl", bufs=6))
        whfpool = ent(tc.tile_pool(name="whf", bufs=6))
        attpool = ent(tc.tile_pool(name="att", bufs=6))
        f1bpool = ent(tc.tile_pool(name="f1b", bufs=8))
        f12pool = ent(tc.tile_pool(name="f12", bufs=8))
        epipool = ent(tc.tile_pool(name="epi", bufs=3))
        xmaxpool = ent(tc.tile_pool(name="xmax", bufs=4))
        smallpool = ent(tc.tile_pool(name="small", bufs=4))
        ps_wh = ent(tc.tile_pool(name="ps_wh", bufs=2, space="PSUM"))
        ps_o = ent(tc.tile_pool(name="ps_o", bufs=2, space="PSUM"))
        ps_s = ent(tc.tile_pool(name="ps_s", bufs=1, space="PSUM"))
        ps_misc = ent(tc.tile_pool(name="ps_misc", bufs=3, space="PSUM"))
        dram_in = ent(tc.tile_pool(name="dram_in", bufs=3, space="DRAM"))
        dram_ag = ent(tc.tile_pool(name="dram_ag", bufs=3, space="DRAM"))
        if True:
            # ---- constants / static inputs
            nonce = os.environ.get("KERNEL_NONCE", "0")
            ident = cpool.tile([128, 128], FP32, name=f"ident_{nonce}",
                               tag="ident")
            masks.make_identity(nc, ident[:])
            ones_f = cpool.tile([128, 1], FP32, name="ones_f", tag="ones_f")
            nc.vector.memset(ones_f[:], 1.0)
            ones_b = cpool.tile([128, 1], BF16, name="ones_b", tag="ones_b")
            nc.vector.tensor_copy(ones_b[:], ones_f[:])

            if not ln_trivial:
                g_row = cpool.tile([1, D], FP32, name="g_row", tag="g_row")
                nc.sync.dma_start(g_row[:], lng_in)
                b_row = cpool.tile([1, D], FP32, name="b_row", tag="b_row")
                nc.sync.dma_start(b_row[:], lnb_in)
                g_bc = cpool.tile([128, D], FP32, name="g_bc", tag="g_bc")
                nc.gpsimd.partition_broadcast(g_bc[:], g_row[:])
                b_bc = cpool.tile([128, D], FP32, name="b_bc", tag="b_bc")
                nc.gpsimd.partition_broadcast(b_bc[:], b_row[:])

            w12_sb = cpool.tile([128, NLAYER, R, KJ, 2], FP32R,
                                name="w12_sb", tag="w12_sb")
            nc.sync.dma_start(
                w12_sb[:], w12_in.rearrange("l r (j p) c -> p l r j c", p=128))

            maskT_sb = cpool.tile([128, R, NT, NL], BF16,
                                  name="maskT_sb", tag="maskT_sb")
            for r in range(R):
                nc.sync.dma_start(
                    maskT_sb[:, r, :, :],
                    maskT_in[r].rearrange("(t p) n -> p t n", p=128))

            # per-layer transposed activations: bf16 for the Wh matmul,
            # fp32r for the f12 logit matmul
            xTb = [ppool.tile([128, KJ, NL], BF16, name=f"xTb{l}", tag=f"xTb{l}")
                   for l in range(NLAYER)]
            xTr = [ppool.tile([128, KJ, NL], FP32R, name=f"xTr{l}", tag=f"xTr{l}")
                   for l in range(NLAYER)]
            nc.sync.dma_start(xTr[0][:], x0T_in.rearrange("(j p) n -> p j n", p=128))
            nc.vector.tensor_copy(xTb[0][:], xTr[0][:])

            for L in range(NLAYER):
                # ---- phase 0: attention logit halves f12T, tiny AllGather
                f12_in_d = dram_in.tile([2 * R, NL], FP32, name="f12_in",
                                        tag="f12_in")
                f12_sb = []
                for r in range(R):
                    f12_ps = ps_misc.tile([2, NL], FP32, name="f12_ps",
                                          tag="misc")
                    for j in range(KJ):
                        nc.tensor.matmul(
                            f12_ps[:], w12_sb[:, L, r, j, :], xTr[L][:, j, :],
                            start=(j == 0), stop=(j == KJ - 1))
                    f12 = f12pool.tile([2, NL], FP32, name="f12", tag="f12")
                    nc.vector.tensor_copy(f12[:], f12_ps[:])
                    f12_sb.append(f12)
                    nc.sync.dma_start(f12_in_d[2 * r:2 * r + 2, :], f12[:])
                f12_ag_d = dram_ag.tile([2 * R * N_CORES, NL], FP32,
                                        addr_space="Shared", name="f12_ag",
                                        tag="f12_ag")
                nc.gpsimd.collective_compute(
                    "AllGather", ALU.bypass, replica_groups=RG,
                    ins=[f12_in_d[:].opt()], outs=[f12_ag_d[:].opt()])

                # ---- phase 1+2: Wh_local (bf16), TWO AllGathers per
                # layer (relations 01 then 23) so phase 3 of the first pair
                # overlaps the second collective
                wh_ag = []
                for g in range(2):
                    wh_in_d = dram_in.tile([2 * NL, D], BF16,
                                           name=f"wh_in{g}", tag=f"wh_in{g}")
                    for rr in range(2):
                        r = 2 * g + rr
                        w_sb = wpool.tile([128, KJ, D], BF16, name="w_sb",
                                          tag="w_sb")
                        nc.sync.dma_start(
                            w_sb[:],
                            wts_in[L, r].rearrange("(j p) o -> p j o", p=128))
                        for h in range(2):
                            wh_ps = ps_wh.tile([128, D], FP32, name="wh_ps",
                                               tag="wh_ps")
                            for j in range(KJ):
                                nc.tensor.matmul(
                                    wh_ps[:],
                                    xTb[L][:, j, h * 128:(h + 1) * 128],
                                    w_sb[:, j, :],
                                    start=(j == 0), stop=(j == KJ - 1))
                            whl = whlpool.tile([128, D], BF16, name="whl",
                                               tag="whl")
                            nc.vector.tensor_copy(whl[:], wh_ps[:])
                            nc.sync.dma_start(
                                wh_in_d[rr * NL + h * 128:
                                        rr * NL + (h + 1) * 128, :],
                                whl[:])
                    wh_ag_d = dram_ag.tile([N_CORES * 2 * NL, D], BF16,
                                           addr_space="Shared",
                                           name=f"wh_ag{g}", tag=f"wh_ag{g}")
                    nc.gpsimd.collective_compute(
                        "AllGather", ALU.bypass, replica_groups=RG,
                        ins=[wh_in_d[:].opt()], outs=[wh_ag_d[:].opt()])
                    wh_ag.append(wh_ag_d)

                # f1 broadcast tiles (local rows), f2 per-partition tiles
                f1_bc = []
                for r in range(R):
                    f1b = f1bpool.tile([128, NL], FP32, name="f1b", tag="f1b")
                    nc.gpsimd.partition_broadcast(f1b[:], f12_sb[r][0:1, :])
                    f1_bc.append(f1b)
                f2_sb = smallpool.tile([128, R, 2, N_CORES], FP32, name="f2",
                                       tag="f2")
                for r in range(R):
                    f2rows = smallpool.tile([N_CORES, NL], FP32, name="f2rows",
                                            tag="f2rows")
                    nc.sync.dma_start(f2rows[:],
                                      f12_ag_d[(2 * r + 1)::(2 * R), :])
                    for h in range(2):
                        f2t_ps = ps_misc.tile([128, N_CORES], FP32,
                                              name="f2t_ps", tag="misc")
                        nc.tensor.transpose(
                            f2t_ps[:], f2rows[:, h * 128:(h + 1) * 128],
                            ident[0:N_CORES, 0:N_CORES])
                        nc.vector.tensor_copy(f2_sb[:, r, h, :], f2t_ps[:])

                # ---- phase 3: attention + epilogue per relation
                xmax = [xmaxpool.tile([128, D], FP32, name="xmax", tag="xmax")
                        for _ in range(2)]
                for r in range(R):
                    o_ps = [ps_o.tile([128, D], FP32, name="o_ps", tag="o_ps")
                            for _ in range(2)]
                    s_ps = ps_s.tile([1, NL], FP32, name="s_ps", tag="s_ps")
                    for tp in range(NT // 2):
                        # pair of m-tiles t=2tp, 2tp+1: contiguous 256 rows
                        # of the gathered Wh and a contiguous 512-wide mask
                        row0 = tp * (2 * NL) + (r % 2) * NL
                        whf = whfpool.tile([128, 2, D], BF16, name="whf",
                                           tag="whf")
                        nc.sync.dma_start(
                            whf[:],
                            wh_ag[r // 2][row0:row0 + 2 * 128, :].rearrange(
                                "(u p) d -> p u d", p=128))
                        z2 = attpool.tile([128, 2, NL], FP32, name="z2",
                                          tag="z2")
                        for hh in range(2):
                            nc.scalar.activation(z2[:, hh, :], f1_bc[r][:],
                                                 AF.Prelu,
                                                 bias=f2_sb[:, r, hh,
                                                            tp:tp + 1],
                                                 alpha=ALPHA)
                        e2 = attpool.tile([128, 2 * NL], BF16, name="e2",
                                          tag="e2")
                        nc.scalar.activation(e2[:], z2[:].rearrange("p u n -> p (u n)"), AF.Exp)
                        pt = attpool.tile([128, 2 * NL], BF16, name="pt",
                                          tag="pt")
                        nc.gpsimd.tensor_mul(
                            pt[:], e2[:],
                            maskT_sb[:, r, 2 * tp:2 * tp + 2, :].rearrange(
                                "p t n -> p (t n)"))
                        for u in range(2):
                            for h in range(2):
                                nc.tensor.matmul(
                                    o_ps[h][:],
                                    pt[:, u * NL + h * 128:
                                       u * NL + (h + 1) * 128],
                                    whf[:, u, :],
                                    start=(tp == 0 and u == 0),
                                    stop=(tp == NT // 2 - 1 and u == 1))
                        for u in range(2):
                            nc.tensor.matmul(
                                s_ps[:], ones_b[:],
                                pt[:, u * NL:(u + 1) * NL],
                                start=(tp == 0 and u == 0),
                                stop=(tp == NT // 2 - 1 and u == 1))

                    rs_row = smallpool.tile([1, NL], FP32, name="rs_row",
                                            tag="rs_row")
                    nc.vector.reciprocal(rs_row[:], s_ps[:])
                    for h in range(2):
                        rs_ps = ps_misc.tile([128, 1], FP32, name="rs_ps",
                                             tag="misc")
                        nc.tensor.transpose(rs_ps[:],
                                            rs_row[:, h * 128:(h + 1) * 128],
                                            ident[0:1, 0:1])
                        rs_col = smallpool.tile([128, 1], FP32, name="rs_col",
                                                tag="rs_col")
                        nc.vector.tensor_copy(rs_col[:], rs_ps[:])

                        xo = epipool.tile([128, D], FP32, name="xo", tag="xo")
                        nc.vector.tensor_scalar_mul(xo[:], o_ps[h][:],
                                                    rs_col[:, 0:1])
                        # elu(xo) + 1 = relu(xo) + exp(min(xo, 0))
                        sum1 = smallpool.tile([128, 1], FP32, name="sum1",
                                              tag="sum1")
                        r1 = epipool.tile([128, D], FP32, name="r1", tag="r1")
                        nc.vector.tensor_scalar(r1[:], xo[:], 0.0, 0.0,
                                                op0=ALU.max, op1=ALU.add,
                                                accum_out=sum1[:])
                        xm = epipool.tile([128, D], FP32, name="xm", tag="xm")
                        nc.vector.tensor_scalar_min(xm[:], xo[:], 0.0)
                        sum2 = smallpool.tile([128, 1], FP32, name="sum2",
                                              tag="sum2")
                        ce = epipool.tile([128, D], FP32, name="ce", tag="ce")
                        nc.scalar.activation(ce[:], xm[:], AF.Exp,
                                             accum_out=sum2[:])
                        xe = epipool.tile([128, D], FP32, name="xe", tag="xe")
                        nc.vector.tensor_add(xe[:], r1[:], ce[:])

                        if L < NLAYER - 1:
                            # layernorm over d (shift-invariant: xe = elu + 1)
                            musum = smallpool.tile([128, 1], FP32, name="musum",
                                                   tag="musum")
                            nc.vector.tensor_add(musum[:], sum1[:], sum2[:])
                            negmu = smallpool.tile([128, 1], FP32, name="negmu",
                                                   tag="negmu")
                            nc.vector.tensor_scalar_mul(negmu[:], musum[:],
                                                        -1.0 / D)
                            ctr = epipool.tile([128, D], FP32, name="ctr",
                                               tag="ctr")
                            nc.vector.tensor_scalar_add(ctr[:], xe[:],
                                                        negmu[:, 0:1])
                            ss = smallpool.tile([128, 1], FP32, name="ss",
                                                tag="ss")
                            sq = epipool.tile([128, D], FP32, name="sq",
                                              tag="sq")
                            nc.vector.scalar_tensor_tensor(
                                sq[:], ctr[:], 1.0, ctr[:],
                                op0=ALU.mult, op1=ALU.mult, accum_out=ss[:])
                            vt = smallpool.tile([128, 1], FP32, name="vt",
                                                tag="vt")
                            nc.vector.tensor_scalar(vt[:], ss[:], 1.0 / D, EPS,
                                                    op0=ALU.mult, op1=ALU.add)
                            lnv = smallpool.tile([128, 1], FP32, name="lnv",
                                                 tag="lnv")
                            nc.scalar.activation(lnv[:], vt[:], AF.Ln)
                            rstd = smallpool.tile([128, 1], FP32, name="rstd",
                                                  tag="rstd")
                            nc.scalar.activation(rstd[:], lnv[:], AF.Exp,
                                                 scale=-0.5)
                            if ln_trivial:
                                if r == 0:
                                    nc.vector.tensor_scalar_mul(
                                        xmax[h][:], ctr[:], rstd[:, 0:1])
                                else:
                                    yv = epipool.tile([128, D], FP32,
                                                      name="yv", tag="yv")
                                    nc.vector.tensor_scalar_mul(
                                        yv[:], ctr[:], rstd[:, 0:1])
                                    nc.vector.tensor_max(xmax[h][:],
                                                         xmax[h][:], yv[:])
                            else:
                                yg = epipool.tile([128, D], FP32, name="yg",
                                                  tag="yg")
                                nc.vector.scalar_tensor_tensor(
                                    yg[:], ctr[:], rstd[:, 0:1], g_bc[:],
                                    op0=ALU.mult, op1=ALU.mult)
                                if r == 0:
                                    nc.vector.tensor_add(xmax[h][:], yg[:],
                                                         b_bc[:])
                                else:
                                    yb = epipool.tile([128, D], FP32,
                                                      name="yb", tag="yb")
                                    nc.vector.tensor_add(yb[:], yg[:], b_bc[:])
                                    nc.vector.tensor_max(xmax[h][:],
                                                         xmax[h][:], yb[:])
                        else:
                            if r == 0:
                                nc.vector.tensor_scalar_add(xmax[h][:], xe[:],
                                                            -1.0)
                            else:
                                ym = epipool.tile([128, D], FP32, name="ym",
                                                  tag="ym")
                                nc.vector.tensor_scalar_add(ym[:], xe[:], -1.0)
                                nc.vector.tensor_max(xmax[h][:], xmax[h][:],
                                                     ym[:])

                # ---- phase 4: outputs / transpose for next layer
                if L == 1:
                    for h in range(2):
                        nc.sync.dma_start(xenc_out[h * 128:(h + 1) * 128, :],
                                          xmax[h][:])
                if L == NLAYER - 1:
                    for h in range(2):
                        nc.sync.dma_start(xd_out[h * 128:(h + 1) * 128, :],
                                          xmax[h][:])
                else:
                    for h in range(2):
                        for j in range(KJ):
                            tr_ps = ps_misc.tile([128, 128], FP32,
                                                 name="tr_ps", tag="misc")
                            nc.tensor.transpose(
                                tr_ps[:], xmax[h][:, j * 128:(j + 1) * 128],
                                ident[:])
                            nc.vector.tensor_copy(
                                xTr[L + 1][:, j, h * 128:(h + 1) * 128],
                                tr_ps[:])
                            nc.vector.tensor_copy(
                                xTb[L + 1][:, j, h * 128:(h + 1) * 128],
                                tr_ps[:])

    nc.compile()
    return nc


def _get_program(ln_trivial):
    if ln_trivial not in _PROGRAMS:
        _PROGRAMS[ln_trivial] = _build_program(ln_trivial)
    return _PROGRAMS[ln_trivial]


def _softmax(x, axis=-1):
    m = x.max(axis=axis, keepdims=True)
    e = np.exp(x - m)
    return e / e.sum(axis=axis, keepdims=True)


def _run_device(x0, rel_adj, W_all, w12, ln_g, ln_b, trace=False):
    import ml_dtypes
    from concourse import bass_utils

    ln_g = np.ascontiguousarray(ln_g.reshape(1, D), dtype=np.float32)
    ln_b = np.ascontiguousarray(ln_b.reshape(1, D), dtype=np.float32)
    ln_trivial = bool(np.all(ln_g == 1.0) and np.all(ln_b == 0.0))
    nc = _get_program(ln_trivial)

    maskT = (rel_adj > 0).transpose(0, 2, 1)  # [R, m, n]
    wts = np.ascontiguousarray(W_all).astype(ml_dtypes.bfloat16)
    w12 = np.ascontiguousarray(w12, dtype=np.float32)

    in_maps = []
    for c in range(N_CORES):
        sl = slice(c * NL, (c + 1) * NL)
        in_maps.append({
            "x0T": np.ascontiguousarray(x0[sl].T, dtype=np.float32),
            "maskT": np.ascontiguousarray(maskT[:, :, sl]).astype(
                ml_dtypes.bfloat16),
            "wts": wts,
            "w12": w12,
            "lng": ln_g,
            "lnb": ln_b,
        })
    res = bass_utils.run_bass_kernel_spmd(
        nc, in_maps, core_ids=list(range(N_CORES)), trace=trace)
    xenc = np.concatenate([res.results[c]["xenc"] for c in range(N_CORES)], 0)
    xd = np.concatenate([res.results[c]["xd"] for c in range(N_CORES)], 0)
    return xenc, xd, res


def kernel(x, rel_adj, qembedding1, qnode_idx, choices_nodes_idx,
           Wc, bc, wq, enc_W, enc_a1, enc_a2, dec_W, dec_a1, dec_a2,
           ln_g, ln_b, Wqc, bqc, Wout, bout, _trace=False):
    x = np.asarray(x, dtype=np.float32)
    qembedding1 = np.asarray(qembedding1, dtype=np.float32)

    # QuestionLayer + question-gated node embeddings (host, tiny)
    qemb = _softmax(qembedding1 @ np.asarray(wq, np.float32))[None, :] @ qembedding1
    qemb = qemb[0]
    v = np.asarray(Wc, np.float32).T @ qemb
    beta = np.asarray(bc, np.float32) @ qemb
    p = 1.0 / (1.0 + np.exp(-(x @ v + beta)))[:, None]
    x0 = p * x + (1.0 - p) * qemb[None, :]

    # stack per-layer weights: enc h=0,1 then dec i=0,1
    W_all = np.stack([enc_W[0], enc_W[1], dec_W[0], dec_W[1]], 0)
    a1_all = np.stack([enc_a1[0], enc_a1[1], dec_a1[0], dec_a1[1]], 0)
    a2_all = np.stack([enc_a2[0], enc_a2[1], dec_a2[0], dec_a2[1]], 0)
    w1 = np.einsum("lrio,lro->lri", W_all, a1_all)
    w2 = np.einsum("lrio,lro->lri", W_all, a2_all)
    w12 = np.stack([w1, w2], axis=-1)  # [L, R, D, 2]

    xenc, xd, res = _run_device(x0, np.asarray(rel_adj), W_all, w12,
                                np.asarray(ln_g), np.asarray(ln_b),
                                trace=_trace)

    # QuestionChoiceLayer (host, tiny)
    cn = xenc[np.asarray(choices_nodes_idx)]              # [5, C, D]
    qproj = qembedding1.mean(0) @ np.asarray(Wqc, np.float32) + np.asarray(bqc, np.float32)
    sc = _softmax(cn @ qproj, axis=-1)                    # [5, C]
    cembed = np.einsum("kc,kcd->kd", sc, cn)              # [5, D]
    logit = cembed @ np.asarray(Wout, np.float32) + np.asarray(bout, np.float32)
    m = logit.max(axis=0, keepdims=True)
    cout = logit - (m + np.log(np.exp(logit - m).sum(axis=0, keepdims=True)))

    if _trace:
        return (cout, xd), res
    return cout, xd
